# revision 17
# baseline (speedup 1.0000x reference)
"""Trainium2 Bass kernel for nn_CorrNet (e3nn-style equivariant MLP + tensor-product head).

Contract: kernel(**inputs) takes the FULL unsharded inputs (as produced by
setup_inputs()) and returns the FULL [N, 1] float32 output.

Strategy (pure data parallel over the atom axis N, 8 NeuronCores):
 - Host: fold every static scalar (1/sqrt(M), 1/sqrt(K), act norms, tp norm,
   output_scale, input_shift) into the weights; eigendecompose the symmetric
   tensor-product forms so the quadratic head becomes
   y = sum_e lam0_e (Q0^T zs)_e^2 + sum_{i,e} lam1_e (Q1^T zv_i)_e^2,
   i.e. pure matmuls + Square activations + a lambda-weighted partition
   reduction (one more matmul with a [128,2] stationary operand).
 - Host: re-layout x into feature-major fp16 arrays per core:
   xu [128, n] (0e block, pre-scaled by C_SILU) and xv [128, 3n/2]
   (1e block, per 1024-sample pair: 3 components x 512 cols, with the
   even tile's 64 rows on partitions 0:64 and the odd tile's on 64:128).
 - Device: inputs are bulk-DMAed into resident SBUF tiles up front (16
   chunked DMAs) and updated in place by the resnet; per 1024-sample
   iteration: 23 matmuls, silu/relu/square on ScalarE, gate multiplies +
   vector residual add on VectorE, scalar residual add on GpSimd, and the
   [2,512] result DMAed straight out of PSUM.

Everything is exact algebra up to fp16 storage rounding of activations and
weights; all accumulation is fp32.
"""

import numpy as np

# ---- problem constants (hardcoded per contest contract) ----
M, K, T = 128, 64, 64
N_TOTAL = 131072
N_CORES = 8
NC_SAMP = N_TOTAL // N_CORES  # 16384 samples per core
NT = 512                      # samples per tile
NPAIR = NC_SAMP // (2 * NT)   # 16 iterations of 1024 samples

C_SILU = 0.5964692111226791
C_RELU = 0.7071067811865186
INV_SQRT_M = float(1.0 / np.sqrt(M))
INV_SQRT_K = float(1.0 / np.sqrt(K))
INV_SQRT_3 = float(1.0 / np.sqrt(3.0))
TP_NORM = float(1.0 / np.sqrt(2.0 * T * T))

# weight-concat column offsets (fp16 [128, NW])
_OFF_LS = (0, 320)
_OFF_LG = (128, 448)
_OFF_BD = (192, 512)
_OFF_LR0 = 640
_OFF_LR1BD = 704
# paired-reduce lhsT columns [128, 2] each
_OFF_RP = 832   # [lam0; 0], [0; lam0]
_OFF_RQ = 834   # [lam1; 0], [0; lam1]
NW = 836
NB = 5  # f32 bias columns: BS1, BS2, BG1dup, BG2dup, BRAdup

# ---- engine assignment knobs ----
UADD_ENGINE = "gpsimd"   # "gpsimd" | "dve"   (scalar-block residual add)
RELU_ENGINE = "scalar"   # "scalar" | "dve"   (gate relu)

_CACHE: dict = {}


def _build_module(n_samp: int, n_pair: int):
    """Build + compile the Bass/Tile module for one core (n_samp = n_pair*1024)."""
    from contextlib import ExitStack

    import concourse.bass as bass
    import concourse.tile as tile
    from concourse import bacc, mybir

    f16 = mybir.dt.float16
    f32 = mybir.dt.float32
    AF = mybir.ActivationFunctionType

    nc = bacc.Bacc(
        "TRN2",
        target_bir_lowering=False,
        debug=False,
        enable_asserts=False,
        num_devices=N_CORES,
    )
    nv = 3 * n_samp // 2
    xu = nc.dram_tensor("xu", [128, n_samp], f16, kind="ExternalInput").ap()
    xv = nc.dram_tensor("xv", [128, nv], f16, kind="ExternalInput").ap()
    wcat = nc.dram_tensor("wcat", [128, NW], f16, kind="ExternalInput").ap()
    bcat = nc.dram_tensor("bcat", [128, NB], f32, kind="ExternalInput").ap()
    n_grp_y = (n_samp // 1024 + 2) // 3
    y = nc.dram_tensor("y", [128, n_grp_y * NT], f32, kind="ExternalOutput").ap()

    with tile.TileContext(nc) as tc, ExitStack() as ctx:
        wpool = ctx.enter_context(tc.tile_pool(name="w", bufs=1))
        iopool = ctx.enter_context(tc.tile_pool(name="io", bufs=1))
        sbp = ctx.enter_context(tc.tile_pool(name="sb", bufs=2))
        psp = ctx.enter_context(tc.tile_pool(name="ps", bufs=1, space="PSUM"))
        ypool = ctx.enter_context(tc.tile_pool(name="yp", bufs=1))

        W = wpool.tile([128, NW], f16, tag="W")
        nc.sync.dma_start(W[:], wcat[:])
        B = wpool.tile([128, NB], f32, tag="B")
        nc.sync.dma_start(B[:], bcat[:])

        XU = iopool.tile([128, n_samp], f16, tag="XU")
        XV = iopool.tile([128, nv], f16, tag="XV")
        # chunked bulk preload (2 pairs' worth per chunk)
        ck_u, ck_v = 2048, 3072
        for k in range(n_samp // ck_u):
            nc.sync.dma_start(XU[:, k * ck_u:(k + 1) * ck_u], xu[:, k * ck_u:(k + 1) * ck_u])
            nc.sync.dma_start(XV[:, k * ck_v:(k + 1) * ck_v], xv[:, k * ck_v:(k + 1) * ck_v])

        # y staging: per 3-iteration group, one [128, 512] block whose
        # partitions {32j, 32j+1} hold (even, odd) results of iteration j
        n_grp = (n_pair + 2) // 3
        Ysb = ypool.tile([128, n_grp * NT], f32, tag="Ysb")
        pyp = ctx.enter_context(tc.tile_pool(name="pyp", bufs=1, space="PSUM"))
        pyt = pyp.tile([128, NT], f32, tag="y")
        nc.vector.memset(pyt[:], 0.0)

        LS = [W[:, _OFF_LS[0]:_OFF_LS[0] + 128], W[:, _OFF_LS[1]:_OFF_LS[1] + 128]]
        LG = [W[:, _OFF_LG[0]:_OFF_LG[0] + 64], W[:, _OFF_LG[1]:_OFF_LG[1] + 64]]
        BD = [W[:, _OFF_BD[0]:_OFF_BD[0] + 128], W[:, _OFF_BD[1]:_OFF_BD[1] + 128]]
        LR0 = W[:, _OFF_LR0:_OFF_LR0 + 64]
        LR1BD = W[:, _OFF_LR1BD:_OFF_LR1BD + 128]
        RP = W[:, _OFF_RP:_OFF_RP + 2]
        RQ = W[:, _OFF_RQ:_OFF_RQ + 2]
        BS = [B[:, 0:1], B[:, 1:2]]
        BG = [B[:, 2:3], B[:, 3:4]]
        BRA = B[:, 4:5]

        for p in range(n_pair):
            u = XU[:, 1024 * p:1024 * (p + 1)]
            ue = u[:, 0:NT]
            uo = u[:, NT:2 * NT]
            v = XV[:, 1536 * p:1536 * (p + 1)]
            v01 = v[:, 0:1024]
            v2 = v[:, 1024:1536]

            for l in range(2):
                sp = psp.tile([128, 1024], f32, tag="sp")
                nc.tensor.matmul(sp[:, 0:NT], LS[l], ue, start=True, stop=True)
                nc.tensor.matmul(sp[:, NT:2 * NT], LS[l], uo, start=True, stop=True)
                pg = psp.tile([128, NT], f32, tag="g", bufs=2)
                nc.tensor.matmul(pg[0:64, :], LG[l], ue, start=True, stop=True)
                nc.tensor.matmul(
                    pg[64:128, :], LG[l], uo,
                    start=True, stop=True, tile_position=(0, 64),
                )
                pv01 = psp.tile([128, 1024], f32, tag="v01")
                nc.tensor.matmul(pv01[:, 0:NT], BD[l], v01[:, 0:NT], start=True, stop=True)
                nc.tensor.matmul(pv01[:, NT:1024], BD[l], v01[:, NT:1024], start=True, stop=True)
                pv2 = psp.tile([128, NT], f32, tag="v2")
                nc.tensor.matmul(pv2[:], BD[l], v2[:], start=True, stop=True)

                ts = sbp.tile([128, 1024], f16, tag="ts")
                nc.scalar.activation(ts[:], sp[:], AF.Silu, bias=BS[l])
                g01 = sbp.tile([128, NT], f16, tag="g01")
                if RELU_ENGINE == "scalar":
                    nc.scalar.activation(g01[:], pg[:], AF.Relu, bias=BG[l])
                else:
                    nc.vector.tensor_scalar_max(g01[:], pg[:], 0.0)

                if UADD_ENGINE == "gpsimd":
                    nc.gpsimd.tensor_add(u, u, ts[:])
                else:
                    nc.vector.tensor_add(u, u, ts[:])

                tv = sbp.tile([128, 1536], f16, tag="tv")
                nc.vector.tensor_mul(tv[:, 0:NT], pv01[:, 0:NT], g01[:])
                nc.vector.tensor_mul(tv[:, NT:1024], pv01[:, NT:1024], g01[:])
                nc.vector.tensor_mul(tv[:, 1024:1536], pv2[:], g01[:])
                nc.vector.tensor_add(v, v, tv[:])

            # output head: P = [Q0^T zs_e ; Q0^T zs_o], Q = Q1^T zv per component
            pP = psp.tile([128, NT], f32, tag="g", bufs=2)
            nc.tensor.matmul(pP[0:64, :], LR0, ue, start=True, stop=True)
            nc.tensor.matmul(
                pP[64:128, :], LR0, uo,
                start=True, stop=True, tile_position=(0, 64),
            )
            sqP = sbp.tile([128, NT], f16, tag="sqP")
            nc.scalar.activation(sqP[:], pP[:], AF.Square, bias=BRA)

            pq01 = psp.tile([128, 1024], f32, tag="v01")
            nc.tensor.matmul(pq01[:, 0:NT], LR1BD, v01[:, 0:NT], start=True, stop=True)
            nc.tensor.matmul(pq01[:, NT:1024], LR1BD, v01[:, NT:1024], start=True, stop=True)
            pq2 = psp.tile([128, NT], f32, tag="v2")
            nc.tensor.matmul(pq2[:], LR1BD, v2[:], start=True, stop=True)
            sqV = sbp.tile([128, 1536], f16, tag="sqV")
            nc.scalar.activation(sqV[:, 0:1024], pq01[:], AF.Square)
            nc.scalar.activation(sqV[:, 1024:1536], pq2[:], AF.Square)

            # lambda-weighted partition reduce -> [2, NT] at partition 32*(p%3)
            j = p % 3
            pysl = pyt[32 * j:32 * j + 2, :]
            nc.tensor.matmul(pysl, RP, sqP[:], start=True, stop=False)
            nc.tensor.matmul(pysl, RQ, sqV[:, 0:NT], start=False, stop=False)
            nc.tensor.matmul(pysl, RQ, sqV[:, NT:1024], start=False, stop=False)
            nc.tensor.matmul(pysl, RQ, sqV[:, 1024:1536], start=False, stop=True)
            if j == 2 or p == n_pair - 1:
                nc.vector.tensor_copy(Ysb[:, bass.ts(p // 3, NT)], pyt[:])

        nc.sync.dma_start(y[:], Ysb[:])

    nc.compile()
    return nc


def _prep_weights(inputs: dict) -> tuple[np.ndarray, np.ndarray]:
    """Fold all scalars into fp16 stationary operands + f32 bias columns."""
    f64 = np.float64
    w0_1 = np.asarray(inputs["w0_1"], f64)
    b0_1 = np.asarray(inputs["b0_1"], f64)
    w1_1 = np.asarray(inputs["w1_1"], f64)
    w0_2 = np.asarray(inputs["w0_2"], f64)
    b0_2 = np.asarray(inputs["b0_2"], f64)
    w1_2 = np.asarray(inputs["w1_2"], f64)
    w0_o = np.asarray(inputs["w0_o"], f64)
    b0_o = np.asarray(inputs["b0_o"], f64)
    w1_o = np.asarray(inputs["w1_o"], f64)
    w_tp0 = np.asarray(inputs["w_tp0"], f64)
    w_tp1 = np.asarray(inputs["w_tp1"], f64)
    gamma = float(np.asarray(inputs["output_scale"]))

    alpha = 1.0 / C_SILU
    im, ik = INV_SQRT_M, INV_SQRT_K

    W0s = 0.5 * (w_tp0 + w_tp0.T) * TP_NORM / gamma
    W1s = 0.5 * (w_tp1 + w_tp1.T) * INV_SQRT_3 * TP_NORM / gamma
    lam0, Q0 = np.linalg.eigh(W0s)
    lam1, Q1 = np.linalg.eigh(W1s)

    wcat = np.zeros((128, NW), np.float16)
    bcat = np.zeros((128, NB), np.float32)
    for l, (w0, b0, w1) in enumerate(((w0_1, b0_1, w1_1), (w0_2, b0_2, w1_2))):
        wcat[:, _OFF_LS[l]:_OFF_LS[l] + 128] = (alpha * im * w0[:, :128]).astype(np.float16)
        wcat[:, _OFF_LG[l]:_OFF_LG[l] + 64] = (alpha * im / C_RELU * w0[:, 128:]).astype(np.float16)
        bd = ik * w1
        wcat[0:64, _OFF_BD[l]:_OFF_BD[l] + 64] = bd.astype(np.float16)
        wcat[64:128, _OFF_BD[l] + 64:_OFF_BD[l] + 128] = bd.astype(np.float16)
        bcat[:, l] = b0[:128].astype(np.float32)
        gate_b = (b0[128:] / C_RELU).astype(np.float32)
        bcat[0:64, 2 + l] = gate_b
        bcat[64:128, 2 + l] = gate_b
    wcat[:, _OFF_LR0:_OFF_LR0 + 64] = (alpha * im * (w0_o @ Q0)).astype(np.float16)
    lr1 = (ik * (w1_o @ Q1)).astype(np.float16)
    wcat[0:64, _OFF_LR1BD:_OFF_LR1BD + 64] = lr1
    wcat[64:128, _OFF_LR1BD + 64:_OFF_LR1BD + 128] = lr1
    # paired reduce weights: col 0 reduces the even-tile half, col 1 the odd half
    wcat[0:64, _OFF_RP] = lam0.astype(np.float16)
    wcat[64:128, _OFF_RP + 1] = lam0.astype(np.float16)
    wcat[0:64, _OFF_RQ] = lam1.astype(np.float16)
    wcat[64:128, _OFF_RQ + 1] = lam1.astype(np.float16)
    br0 = (Q0.T @ b0_o).astype(np.float32)
    bcat[0:64, 4] = br0
    bcat[64:128, 4] = br0
    return wcat, bcat


def _prep_x(x: np.ndarray, shift: np.ndarray, n_samp: int,
            n_cores: int = N_CORES) -> list[tuple[np.ndarray, np.ndarray]]:
    """Per-core feature-major fp16 arrays: xu [128, n], xv [128, 3n/2]."""
    xs_scale = np.float32(C_SILU)
    shift = np.asarray(shift, np.float32)
    n_pair = n_samp // 1024
    out = []
    for c in range(n_cores):
        blk = np.asarray(x[c * n_samp:(c + 1) * n_samp], np.float32) - shift
        xu = np.ascontiguousarray((blk[:, :128] * xs_scale).T.astype(np.float16))
        # vecs: [n, 64, 3] -> comp-major [3, 64, n]
        vv = blk[:, 128:].reshape(n_samp, 64, 3).transpose(2, 1, 0).astype(np.float16)
        # [3, 64, n] -> [3, 64, n_pair, 2, 512]; half 0 = even tile, 1 = odd
        vv = vv.reshape(3, 64, n_pair, 2, NT)
        xvh = np.empty((2, 64, n_pair, 3, NT), np.float16)
        xvh[0] = vv[:, :, :, 0, :].transpose(1, 2, 0, 3)   # even rows 0:64
        xvh[1] = vv[:, :, :, 1, :].transpose(1, 2, 0, 3)   # odd rows 64:128
        xv = xvh.reshape(128, n_pair * 3 * NT)
        out.append((xu, np.ascontiguousarray(xv)))
    return out


def _get_module():
    if "nc" not in _CACHE:
        _CACHE["nc"] = _build_module(NC_SAMP, NPAIR)
    return _CACHE["nc"]


def run(inputs: dict, trace: bool = False):
    """Run on 8 NeuronCores; returns (y [N,1] f32, BassKernelResults)."""
    from concourse import bass_utils
    from concourse.bass_interp import get_hw_module

    nc = _get_module()
    wcat, bcat = _prep_weights(inputs)
    xs = _prep_x(np.asarray(inputs["x"]), np.asarray(inputs["input_shift"]), NC_SAMP)
    in_maps = [
        {"xu": xs[c][0], "xv": xs[c][1], "wcat": wcat, "bcat": bcat}
        for c in range(N_CORES)
    ]

    old_m = nc.m
    nc.m = get_hw_module(nc.m)
    try:
        res = bass_utils.run_bass_kernel_spmd(
            nc,
            in_maps,
            core_ids=list(range(N_CORES)),
            trace=trace,
        )
    finally:
        nc.m = old_m

    # de-interleave: y dram is [128, n_grp*512]; group q col-block 512 holds
    # pair p=3q+j at partitions {32j: even tile, 32j+1: odd tile}
    parts = []
    for c in range(N_CORES):
        yc = res.results[c]["y"]
        arr = np.empty((2 * NPAIR, NT), np.float32)
        for p in range(NPAIR):
            q, j = divmod(p, 3)
            arr[2 * p] = yc[32 * j, NT * q:NT * (q + 1)]
            arr[2 * p + 1] = yc[32 * j + 1, NT * q:NT * (q + 1)]
        parts.append(arr.reshape(-1))
    yfull = np.concatenate(parts)
    return yfull.astype(np.float32)[:, None], res


def kernel(**inputs) -> np.ndarray:
    y, _ = run(inputs, trace=False)
    return y


# revision 24
# speedup vs baseline: 1.1109x; 1.1109x over previous
"""Trainium2 Bass kernel for nn_CorrNet (e3nn-style equivariant MLP + tensor-product head).

Contract: kernel(**inputs) takes the FULL unsharded inputs (as produced by
setup_inputs()) and returns the FULL [N, 1] float32 output.

Strategy (pure data parallel over the atom axis N, 8 NeuronCores):
 - Host: fold every static scalar (1/sqrt(M), 1/sqrt(K), act norms, tp norm,
   output_scale, input_shift) into the weights; eigendecompose the symmetric
   tensor-product forms so the quadratic head becomes
   y = sum_e lam0_e (Q0^T zs)_e^2 + sum_{i,e} lam1_e (Q1^T zv_i)_e^2,
   i.e. pure matmuls + Square activations + a lambda-weighted partition
   reduction (one more matmul with a [128,2] stationary operand).
 - Host: re-layout x into feature-major fp16 arrays per core:
   xu [128, n] (0e block, pre-scaled by C_SILU) and xv [128, 3n/2]
   (1e block, per 1024-sample pair: 3 components x 512 cols, with the
   even tile's 64 rows on partitions 0:64 and the odd tile's on 64:128).
 - Device: inputs are bulk-DMAed into resident SBUF tiles up front (16
   chunked DMAs) and updated in place by the resnet; per 1024-sample
   iteration: 23 matmuls, silu/relu/square on ScalarE, gate multiplies +
   vector residual add on VectorE, scalar residual add on GpSimd, and the
   [2,512] result DMAed straight out of PSUM.

Everything is exact algebra up to fp16 storage rounding of activations and
weights; all accumulation is fp32.
"""

import numpy as np

# ---- problem constants (hardcoded per contest contract) ----
M, K, T = 128, 64, 64
N_TOTAL = 131072
N_CORES = 8
NC_SAMP = N_TOTAL // N_CORES  # 16384 samples per core
NT = 512                      # samples per tile
NPAIR = NC_SAMP // (2 * NT)   # 16 iterations of 1024 samples

C_SILU = 0.5964692111226791
C_RELU = 0.7071067811865186
INV_SQRT_M = float(1.0 / np.sqrt(M))
INV_SQRT_K = float(1.0 / np.sqrt(K))
INV_SQRT_3 = float(1.0 / np.sqrt(3.0))
TP_NORM = float(1.0 / np.sqrt(2.0 * T * T))

# weight-concat column offsets (fp16 [128, NW])
_OFF_LS = (0, 320)
_OFF_LG = (128, 448)
_OFF_BD = (192, 512)
_OFF_LR0 = 640
_OFF_LR1BD = 704
# paired-reduce lhsT columns [128, 2] each
_OFF_RP = 832   # [lam0; 0], [0; lam0]
_OFF_RQ = 834   # [lam1; 0], [0; lam1]
NW = 836
NB = 5  # f32 bias columns: BS1, BS2, BG1dup, BG2dup, BRAdup

# ---- engine assignment knobs ----
UADD_ENGINE = "gpsimd"   # "gpsimd" | "dve"   (scalar-block residual add)
RELU_ENGINE = "scalar"   # "scalar" | "dve"   (gate relu)
YCOPY_ENGINE = "dve"     # "dve" | "scalar"   ([2,512] PSUM -> SBUF y stage)

_CACHE: dict = {}


def _build_module(n_samp: int, n_pair: int):
    """Build + compile the Bass/Tile module for one core (n_samp = n_pair*1024)."""
    from contextlib import ExitStack

    import concourse.bass as bass
    import concourse.tile as tile
    from concourse import bacc, mybir

    f16 = mybir.dt.float16
    f32 = mybir.dt.float32
    AF = mybir.ActivationFunctionType

    nc = bacc.Bacc(
        "TRN2",
        target_bir_lowering=False,
        debug=False,
        enable_asserts=False,
        num_devices=N_CORES,
    )
    nv = 3 * n_samp // 2
    xu = nc.dram_tensor("xu", [128, n_samp], f16, kind="ExternalInput").ap()
    xv = nc.dram_tensor("xv", [128, nv], f16, kind="ExternalInput").ap()
    wcat = nc.dram_tensor("wcat", [128, NW], f16, kind="ExternalInput").ap()
    bcat = nc.dram_tensor("bcat", [128, NB], f32, kind="ExternalInput").ap()
    y = nc.dram_tensor("y", [2, n_samp // 2], f32, kind="ExternalOutput").ap()

    with tile.TileContext(nc) as tc, ExitStack() as ctx:
        wpool = ctx.enter_context(tc.tile_pool(name="w", bufs=1))
        iopool = ctx.enter_context(tc.tile_pool(name="io", bufs=1))
        sbp = ctx.enter_context(tc.tile_pool(name="sb", bufs=2))
        psp = ctx.enter_context(tc.tile_pool(name="ps", bufs=1, space="PSUM"))
        ypool = ctx.enter_context(tc.tile_pool(name="yp", bufs=1))

        W = wpool.tile([128, NW], f16, tag="W")
        nc.sync.dma_start(W[:], wcat[:])
        B = wpool.tile([128, NB], f32, tag="B")
        nc.sync.dma_start(B[:], bcat[:])

        XU = iopool.tile([128, n_samp], f16, tag="XU")
        XV = iopool.tile([128, nv], f16, tag="XV")
        # chunked bulk preload (2 pairs' worth per chunk)
        ck_u, ck_v = 2048, 3072
        for k in range(n_samp // ck_u):
            nc.sync.dma_start(XU[:, k * ck_u:(k + 1) * ck_u], xu[:, k * ck_u:(k + 1) * ck_u])
            nc.sync.dma_start(XV[:, k * ck_v:(k + 1) * ck_v], xv[:, k * ck_v:(k + 1) * ck_v])

        LS =[W[:, _OFF_LS[0]:_OFF_LS[0] + 128], W[:, _OFF_LS[1]:_OFF_LS[1] + 128]]
        LG = [W[:, _OFF_LG[0]:_OFF_LG[0] + 64], W[:, _OFF_LG[1]:_OFF_LG[1] + 64]]
        BD = [W[:, _OFF_BD[0]:_OFF_BD[0] + 128], W[:, _OFF_BD[1]:_OFF_BD[1] + 128]]
        LR0 = W[:, _OFF_LR0:_OFF_LR0 + 64]
        LR1BD = W[:, _OFF_LR1BD:_OFF_LR1BD + 128]
        RP = W[:, _OFF_RP:_OFF_RP + 2]
        RQ = W[:, _OFF_RQ:_OFF_RQ + 2]
        BS = [B[:, 0:1], B[:, 1:2]]
        BG = [B[:, 2:3], B[:, 3:4]]
        BRA = B[:, 4:5]

        def uadd(u_sl, ts_t):
            if UADD_ENGINE == "gpsimd":
                nc.gpsimd.tensor_add(u_sl, u_sl, ts_t[:])
            else:
                nc.vector.tensor_add(u_sl, u_sl, ts_t[:])

        # deferred reduce state from the previous iteration (1-iter skew)
        pend = None

        def emit_reduce(pd):
            # reduce of a prior iteration: 4 chained matmuls into its v2y
            # tile's partitions [0:2], then a [2,512] copy to Ysb
            p0, sqP0, sqV0, pv2y0 = pd
            ysl = pv2y0[0:2, :]
            nc.tensor.matmul(ysl, RP, sqP0[:], start=True, stop=False)
            nc.tensor.matmul(ysl, RQ, sqV0[:, 0:NT], start=False, stop=False)
            nc.tensor.matmul(ysl, RQ, sqV0[:, NT:1024], start=False, stop=False)
            nc.tensor.matmul(ysl, RQ, sqV0[:, 1024:1536], start=False, stop=True)
            yt = sbp.tile([2, NT], f32, tag="yt")
            if YCOPY_ENGINE == "scalar":
                nc.scalar.copy(yt[:], ysl)
            else:
                nc.vector.tensor_copy(yt[:], ysl)
            nc.sync.dma_start(y[0:2, bass.ts(p0, NT)], yt[:])

        for p in range(n_pair):
            u = XU[:, 1024 * p:1024 * (p + 1)]
            ue = u[:, 0:NT]
            uo = u[:, NT:2 * NT]
            v = XV[:, 1536 * p:1536 * (p + 1)]
            v01 = v[:, 0:1024]
            v2 = v[:, 1024:1536]

            ts_l = []
            pv2y = None
            for l in range(2):
                sp = psp.tile([128, 1024], f32, tag="sp")
                nc.tensor.matmul(sp[:, 0:NT], LS[l], ue, start=True, stop=True)
                nc.tensor.matmul(sp[:, NT:2 * NT], LS[l], uo, start=True, stop=True)
                pg = psp.tile([128, NT], f32, tag="gP")
                nc.tensor.matmul(pg[0:64, :], LG[l], ue, start=True, stop=True)
                nc.tensor.matmul(
                    pg[64:128, :], LG[l], uo,
                    start=True, stop=True, tile_position=(0, 64),
                )
                pv01 = psp.tile([128, 1024], f32, tag="v01", bufs=2)
                nc.tensor.matmul(pv01[:, 0:NT], BD[l], v01[:, 0:NT], start=True, stop=True)
                nc.tensor.matmul(pv01[:, NT:1024], BD[l], v01[:, NT:1024], start=True, stop=True)

                if l == 0 and pend is not None:
                    emit_reduce(pend)   # PE filler + frees old v2y tile

                pv2 = psp.tile([128, NT], f32, tag="v2y")
                nc.tensor.matmul(pv2[:], BD[l], v2[:], start=True, stop=True)

                ts = sbp.tile([128, 1024], f16, tag="ts")
                nc.scalar.activation(ts[:], sp[:], AF.Silu, bias=BS[l])
                g01 = sbp.tile([128, NT], f16, tag="g01")
                nc.scalar.activation(g01[:], pg[:], AF.Relu, bias=BG[l])

                uadd(u, ts)

                tv = sbp.tile([128, 1536], f16, tag="tv")
                nc.vector.tensor_mul(tv[:, 0:NT], pv01[:, 0:NT], g01[:])
                nc.vector.tensor_mul(tv[:, NT:1024], pv01[:, NT:1024], g01[:])
                nc.vector.tensor_mul(tv[:, 1024:1536], pv2[:], g01[:])
                nc.vector.tensor_add(v, v, tv[:])

            # output head
            pP = psp.tile([128, NT], f32, tag="gP")
            nc.tensor.matmul(pP[0:64, :], LR0, ue, start=True, stop=True)
            nc.tensor.matmul(
                pP[64:128, :], LR0, uo,
                start=True, stop=True, tile_position=(0, 64),
            )
            sqP = sbp.tile([128, NT], f16, tag="sqP")
            nc.scalar.activation(sqP[:], pP[:], AF.Square, bias=BRA)

            pq01 = psp.tile([128, 1024], f32, tag="v01", bufs=2)
            nc.tensor.matmul(pq01[:, 0:NT], LR1BD, v01[:, 0:NT], start=True, stop=True)
            nc.tensor.matmul(pq01[:, NT:1024], LR1BD, v01[:, NT:1024], start=True, stop=True)
            pv2y = psp.tile([128, NT], f32, tag="v2y")
            nc.tensor.matmul(pv2y[:], LR1BD, v2[:], start=True, stop=True)
            sqV = sbp.tile([128, 1536], f16, tag="sqV")
            nc.scalar.activation(sqV[:, 0:1024], pq01[:], AF.Square)
            nc.scalar.activation(sqV[:, 1024:1536], pv2y[:], AF.Square)

            pend = (p, sqP, sqV, pv2y)

        emit_reduce(pend)

    nc.compile()
    return nc


def _prep_weights(inputs: dict) -> tuple[np.ndarray, np.ndarray]:
    """Fold all scalars into fp16 stationary operands + f32 bias columns."""
    f64 = np.float64
    w0_1 = np.asarray(inputs["w0_1"], f64)
    b0_1 = np.asarray(inputs["b0_1"], f64)
    w1_1 = np.asarray(inputs["w1_1"], f64)
    w0_2 = np.asarray(inputs["w0_2"], f64)
    b0_2 = np.asarray(inputs["b0_2"], f64)
    w1_2 = np.asarray(inputs["w1_2"], f64)
    w0_o = np.asarray(inputs["w0_o"], f64)
    b0_o = np.asarray(inputs["b0_o"], f64)
    w1_o = np.asarray(inputs["w1_o"], f64)
    w_tp0 = np.asarray(inputs["w_tp0"], f64)
    w_tp1 = np.asarray(inputs["w_tp1"], f64)
    gamma = float(np.asarray(inputs["output_scale"]))

    alpha = 1.0 / C_SILU
    im, ik = INV_SQRT_M, INV_SQRT_K

    W0s = 0.5 * (w_tp0 + w_tp0.T) * TP_NORM / gamma
    W1s = 0.5 * (w_tp1 + w_tp1.T) * INV_SQRT_3 * TP_NORM / gamma
    lam0, Q0 = np.linalg.eigh(W0s)
    lam1, Q1 = np.linalg.eigh(W1s)

    wcat = np.zeros((128, NW), np.float16)
    bcat = np.zeros((128, NB), np.float32)
    for l, (w0, b0, w1) in enumerate(((w0_1, b0_1, w1_1), (w0_2, b0_2, w1_2))):
        wcat[:, _OFF_LS[l]:_OFF_LS[l] + 128] = (alpha * im * w0[:, :128]).astype(np.float16)
        wcat[:, _OFF_LG[l]:_OFF_LG[l] + 64] = (alpha * im / C_RELU * w0[:, 128:]).astype(np.float16)
        bd = ik * w1
        wcat[0:64, _OFF_BD[l]:_OFF_BD[l] + 64] = bd.astype(np.float16)
        wcat[64:128, _OFF_BD[l] + 64:_OFF_BD[l] + 128] = bd.astype(np.float16)
        bcat[:, l] = b0[:128].astype(np.float32)
        gate_b = (b0[128:] / C_RELU).astype(np.float32)
        bcat[0:64, 2 + l] = gate_b
        bcat[64:128, 2 + l] = gate_b
    wcat[:, _OFF_LR0:_OFF_LR0 + 64] = (alpha * im * (w0_o @ Q0)).astype(np.float16)
    lr1 = (ik * (w1_o @ Q1)).astype(np.float16)
    wcat[0:64, _OFF_LR1BD:_OFF_LR1BD + 64] = lr1
    wcat[64:128, _OFF_LR1BD + 64:_OFF_LR1BD + 128] = lr1
    # paired reduce weights: col 0 reduces the even-tile half, col 1 the odd half
    wcat[0:64, _OFF_RP] = lam0.astype(np.float16)
    wcat[64:128, _OFF_RP + 1] = lam0.astype(np.float16)
    wcat[0:64, _OFF_RQ] = lam1.astype(np.float16)
    wcat[64:128, _OFF_RQ + 1] = lam1.astype(np.float16)
    br0 = (Q0.T @ b0_o).astype(np.float32)
    bcat[0:64, 4] = br0
    bcat[64:128, 4] = br0
    return wcat, bcat


def _prep_x(x: np.ndarray, shift: np.ndarray, n_samp: int,
            n_cores: int = N_CORES) -> list[tuple[np.ndarray, np.ndarray]]:
    """Per-core feature-major fp16 arrays: xu [128, n], xv [128, 3n/2]."""
    xs_scale = np.float32(C_SILU)
    shift = np.asarray(shift, np.float32)
    n_pair = n_samp // 1024
    out = []
    for c in range(n_cores):
        blk = np.asarray(x[c * n_samp:(c + 1) * n_samp], np.float32) - shift
        xu = np.ascontiguousarray((blk[:, :128] * xs_scale).T.astype(np.float16))
        # vecs: [n, 64, 3] -> comp-major [3, 64, n]
        vv = blk[:, 128:].reshape(n_samp, 64, 3).transpose(2, 1, 0).astype(np.float16)
        # [3, 64, n] -> [3, 64, n_pair, 2, 512]; half 0 = even tile, 1 = odd
        vv = vv.reshape(3, 64, n_pair, 2, NT)
        xvh = np.empty((2, 64, n_pair, 3, NT), np.float16)
        xvh[0] = vv[:, :, :, 0, :].transpose(1, 2, 0, 3)   # even rows 0:64
        xvh[1] = vv[:, :, :, 1, :].transpose(1, 2, 0, 3)   # odd rows 64:128
        xv = xvh.reshape(128, n_pair * 3 * NT)
        out.append((xu, np.ascontiguousarray(xv)))
    return out


def _get_module():
    if "nc" not in _CACHE:
        _CACHE["nc"] = _build_module(NC_SAMP, NPAIR)
    return _CACHE["nc"]


def run(inputs: dict, trace: bool = False):
    """Run on 8 NeuronCores; returns (y [N,1] f32, BassKernelResults)."""
    from concourse import bass_utils
    from concourse.bass_interp import get_hw_module

    nc = _get_module()
    wcat, bcat = _prep_weights(inputs)
    xs = _prep_x(np.asarray(inputs["x"]), np.asarray(inputs["input_shift"]), NC_SAMP)
    in_maps = [
        {"xu": xs[c][0], "xv": xs[c][1], "wcat": wcat, "bcat": bcat}
        for c in range(N_CORES)
    ]

    old_m = nc.m
    nc.m = get_hw_module(nc.m)
    try:
        res = bass_utils.run_bass_kernel_spmd(
            nc,
            in_maps,
            core_ids=list(range(N_CORES)),
            trace=trace,
        )
    finally:
        nc.m = old_m

    # de-interleave: y dram is [2, NC/2]; row 0 = even tiles, row 1 = odd
    parts = []
    for c in range(N_CORES):
        yc = res.results[c]["y"]
        arr = np.empty((2 * NPAIR, NT), np.float32)
        arr[0::2] = yc[0].reshape(NPAIR, NT)
        arr[1::2] = yc[1].reshape(NPAIR, NT)
        parts.append(arr.reshape(-1))
    yfull = np.concatenate(parts)
    return yfull.astype(np.float32)[:, None], res


def kernel(**inputs) -> np.ndarray:
    y, _ = run(inputs, trace=False)
    return y


# revision 25
# speedup vs baseline: 1.1748x; 1.0574x over previous
"""Trainium2 Bass kernel for nn_CorrNet (e3nn-style equivariant MLP + tensor-product head).

Contract: kernel(**inputs) takes the FULL unsharded inputs (as produced by
setup_inputs()) and returns the FULL [N, 1] float32 output.

Strategy (pure data parallel over the atom axis N, 8 NeuronCores):
 - Host: fold every static scalar (1/sqrt(M), 1/sqrt(K), act norms, tp norm,
   output_scale, input_shift) into the weights; eigendecompose the symmetric
   tensor-product forms so the quadratic head becomes
   y = sum_e lam0_e (Q0^T zs)_e^2 + sum_{i,e} lam1_e (Q1^T zv_i)_e^2,
   i.e. pure matmuls + Square activations + a lambda-weighted partition
   reduction (one more matmul with a [128,2] stationary operand).
 - Host: re-layout x into feature-major fp16 arrays per core:
   xu [128, n] (0e block, pre-scaled by C_SILU) and xv [128, 3n/2]
   (1e block, per 1024-sample pair: 3 components x 512 cols, with the
   even tile's 64 rows on partitions 0:64 and the odd tile's on 64:128).
 - Device: inputs are bulk-DMAed into resident SBUF tiles up front (16
   chunked DMAs) and updated in place by the resnet; per 1024-sample
   iteration: 23 matmuls, silu/relu/square on ScalarE, gate multiplies +
   vector residual add on VectorE, scalar residual add on GpSimd, and the
   [2,512] result DMAed straight out of PSUM.

Everything is exact algebra up to fp16 storage rounding of activations and
weights; all accumulation is fp32.
"""

import numpy as np

# ---- problem constants (hardcoded per contest contract) ----
M, K, T = 128, 64, 64
N_TOTAL = 131072
N_CORES = 8
NC_SAMP = N_TOTAL // N_CORES  # 16384 samples per core
NT = 512                      # samples per tile
NPAIR = NC_SAMP // (2 * NT)   # 16 iterations of 1024 samples

C_SILU = 0.5964692111226791
C_RELU = 0.7071067811865186
INV_SQRT_M = float(1.0 / np.sqrt(M))
INV_SQRT_K = float(1.0 / np.sqrt(K))
INV_SQRT_3 = float(1.0 / np.sqrt(3.0))
TP_NORM = float(1.0 / np.sqrt(2.0 * T * T))

# weight-concat column offsets (fp16 [128, NW])
_OFF_LS = (0, 320)
_OFF_LG = (128, 448)
_OFF_BD = (192, 512)
_OFF_LR0 = 640
_OFF_LR1BD = 704
# paired-reduce lhsT columns [128, 2] each
_OFF_RP = 832   # [lam0; 0], [0; lam0]
_OFF_RQ = 834   # [lam1; 0], [0; lam1]
NW = 836
NB = 5  # f32 bias columns: BS1, BS2, BG1dup, BG2dup, BRAdup

# ---- engine assignment knobs ----
UADD_ENGINE = "dve"      # "gpsimd" | "dve"   (scalar-block residual add)
RELU_ENGINE = "scalar"   # "scalar" | "dve"   (gate relu)
YCOPY_ENGINE = "dve"     # "dve" | "scalar"   ([2,512] PSUM -> SBUF y stage)

_CACHE: dict = {}


def _build_module(n_samp: int, n_pair: int):
    """Build + compile the Bass/Tile module for one core (n_samp = n_pair*1024)."""
    from contextlib import ExitStack

    import concourse.bass as bass
    import concourse.tile as tile
    from concourse import bacc, mybir

    f16 = mybir.dt.float16
    f32 = mybir.dt.float32
    AF = mybir.ActivationFunctionType

    nc = bacc.Bacc(
        "TRN2",
        target_bir_lowering=False,
        debug=False,
        enable_asserts=False,
        num_devices=N_CORES,
    )
    nv = 3 * n_samp // 2
    xu = nc.dram_tensor("xu", [128, n_samp], f16, kind="ExternalInput").ap()
    xv = nc.dram_tensor("xv", [128, nv], f16, kind="ExternalInput").ap()
    wcat = nc.dram_tensor("wcat", [128, NW], f16, kind="ExternalInput").ap()
    bcat = nc.dram_tensor("bcat", [128, NB], f32, kind="ExternalInput").ap()
    y = nc.dram_tensor("y", [2, n_samp // 2], f32, kind="ExternalOutput").ap()

    with tile.TileContext(nc) as tc, ExitStack() as ctx:
        wpool = ctx.enter_context(tc.tile_pool(name="w", bufs=1))
        iopool = ctx.enter_context(tc.tile_pool(name="io", bufs=1))
        sbp = ctx.enter_context(tc.tile_pool(name="sb", bufs=2))
        psp = ctx.enter_context(tc.tile_pool(name="ps", bufs=1, space="PSUM"))
        ypool = ctx.enter_context(tc.tile_pool(name="yp", bufs=1))

        W = wpool.tile([128, NW], f16, tag="W")
        nc.sync.dma_start(W[:], wcat[:])
        B = wpool.tile([128, NB], f32, tag="B")
        nc.sync.dma_start(B[:], bcat[:])

        XU = iopool.tile([128, n_samp], f16, tag="XU")
        XV = iopool.tile([128, nv], f16, tag="XV")
        # chunked bulk preload (2 pairs' worth per chunk)
        ck_u, ck_v = 2048, 3072
        for k in range(n_samp // ck_u):
            nc.sync.dma_start(XU[:, k * ck_u:(k + 1) * ck_u], xu[:, k * ck_u:(k + 1) * ck_u])
            nc.sync.dma_start(XV[:, k * ck_v:(k + 1) * ck_v], xv[:, k * ck_v:(k + 1) * ck_v])

        LS =[W[:, _OFF_LS[0]:_OFF_LS[0] + 128], W[:, _OFF_LS[1]:_OFF_LS[1] + 128]]
        LG = [W[:, _OFF_LG[0]:_OFF_LG[0] + 64], W[:, _OFF_LG[1]:_OFF_LG[1] + 64]]
        BD = [W[:, _OFF_BD[0]:_OFF_BD[0] + 128], W[:, _OFF_BD[1]:_OFF_BD[1] + 128]]
        LR0 = W[:, _OFF_LR0:_OFF_LR0 + 64]
        LR1BD = W[:, _OFF_LR1BD:_OFF_LR1BD + 128]
        RP = W[:, _OFF_RP:_OFF_RP + 2]
        RQ = W[:, _OFF_RQ:_OFF_RQ + 2]
        BS = [B[:, 0:1], B[:, 1:2]]
        BG = [B[:, 2:3], B[:, 3:4]]
        BRA = B[:, 4:5]

        def uadd(u_sl, ts_t):
            if UADD_ENGINE == "gpsimd":
                nc.gpsimd.tensor_add(u_sl, u_sl, ts_t[:])
            else:
                nc.vector.tensor_add(u_sl, u_sl, ts_t[:])

        # deferred reduce state from the previous iteration (1-iter skew)
        pend = None

        def emit_reduce(pd):
            # reduce of a prior iteration: 4 chained matmuls into its v2y
            # tile's partitions [0:2], then a [2,512] copy to Ysb
            p0, sqP0, sqV0, pv2y0 = pd
            ysl = pv2y0[0:2, :]
            nc.tensor.matmul(ysl, RP, sqP0[:], start=True, stop=False)
            nc.tensor.matmul(ysl, RQ, sqV0[:, 0:NT], start=False, stop=False)
            nc.tensor.matmul(ysl, RQ, sqV0[:, NT:1024], start=False, stop=False)
            nc.tensor.matmul(ysl, RQ, sqV0[:, 1024:1536], start=False, stop=True)
            yt = sbp.tile([2, NT], f32, tag="yt")
            if YCOPY_ENGINE == "scalar":
                nc.scalar.copy(yt[:], ysl)
            else:
                nc.vector.tensor_copy(yt[:], ysl)
            nc.sync.dma_start(y[0:2, bass.ts(p0, NT)], yt[:])

        for p in range(n_pair):
            u = XU[:, 1024 * p:1024 * (p + 1)]
            ue = u[:, 0:NT]
            uo = u[:, NT:2 * NT]
            v = XV[:, 1536 * p:1536 * (p + 1)]
            v01 = v[:, 0:1024]
            v2 = v[:, 1024:1536]

            ts_l = []
            pv2y = None
            for l in range(2):
                sp = psp.tile([128, 1024], f32, tag="sp")
                nc.tensor.matmul(sp[:, 0:NT], LS[l], ue, start=True, stop=True)
                nc.tensor.matmul(sp[:, NT:2 * NT], LS[l], uo, start=True, stop=True)
                pg = psp.tile([128, NT], f32, tag="gP")
                nc.tensor.matmul(pg[0:64, :], LG[l], ue, start=True, stop=True)
                nc.tensor.matmul(
                    pg[64:128, :], LG[l], uo,
                    start=True, stop=True, tile_position=(0, 64),
                )
                pv01 = psp.tile([128, 1024], f32, tag="v01", bufs=2)
                nc.tensor.matmul(pv01[:, 0:NT], BD[l], v01[:, 0:NT], start=True, stop=True)
                nc.tensor.matmul(pv01[:, NT:1024], BD[l], v01[:, NT:1024], start=True, stop=True)

                if l == 0 and pend is not None:
                    emit_reduce(pend)   # PE filler + frees old v2y tile

                pv2 = psp.tile([128, NT], f32, tag="v2y")
                nc.tensor.matmul(pv2[:], BD[l], v2[:], start=True, stop=True)

                ts = sbp.tile([128, 1024], f16, tag="ts")
                nc.scalar.activation(ts[:], sp[:], AF.Silu, bias=BS[l])
                g01 = sbp.tile([128, NT], f16, tag="g01")
                nc.scalar.activation(g01[:], pg[:], AF.Relu, bias=BG[l])

                uadd(u, ts)

                tv = sbp.tile([128, 1536], f16, tag="tv")
                nc.vector.tensor_mul(tv[:, 0:NT], pv01[:, 0:NT], g01[:])
                nc.vector.tensor_mul(tv[:, NT:1024], pv01[:, NT:1024], g01[:])
                nc.vector.tensor_mul(tv[:, 1024:1536], pv2[:], g01[:])
                nc.vector.tensor_add(v, v, tv[:])

            # output head
            pP = psp.tile([128, NT], f32, tag="gP")
            nc.tensor.matmul(pP[0:64, :], LR0, ue, start=True, stop=True)
            nc.tensor.matmul(
                pP[64:128, :], LR0, uo,
                start=True, stop=True, tile_position=(0, 64),
            )
            sqP = sbp.tile([128, NT], f16, tag="sqP")
            nc.scalar.activation(sqP[:], pP[:], AF.Square, bias=BRA)

            pq01 = psp.tile([128, 1024], f32, tag="v01", bufs=2)
            nc.tensor.matmul(pq01[:, 0:NT], LR1BD, v01[:, 0:NT], start=True, stop=True)
            nc.tensor.matmul(pq01[:, NT:1024], LR1BD, v01[:, NT:1024], start=True, stop=True)
            pv2y = psp.tile([128, NT], f32, tag="v2y")
            nc.tensor.matmul(pv2y[:], LR1BD, v2[:], start=True, stop=True)
            sqV = sbp.tile([128, 1536], f16, tag="sqV")
            nc.scalar.activation(sqV[:, 0:1024], pq01[:], AF.Square)
            nc.scalar.activation(sqV[:, 1024:1536], pv2y[:], AF.Square)

            pend = (p, sqP, sqV, pv2y)

        emit_reduce(pend)

    nc.compile()
    return nc


def _prep_weights(inputs: dict) -> tuple[np.ndarray, np.ndarray]:
    """Fold all scalars into fp16 stationary operands + f32 bias columns."""
    f64 = np.float64
    w0_1 = np.asarray(inputs["w0_1"], f64)
    b0_1 = np.asarray(inputs["b0_1"], f64)
    w1_1 = np.asarray(inputs["w1_1"], f64)
    w0_2 = np.asarray(inputs["w0_2"], f64)
    b0_2 = np.asarray(inputs["b0_2"], f64)
    w1_2 = np.asarray(inputs["w1_2"], f64)
    w0_o = np.asarray(inputs["w0_o"], f64)
    b0_o = np.asarray(inputs["b0_o"], f64)
    w1_o = np.asarray(inputs["w1_o"], f64)
    w_tp0 = np.asarray(inputs["w_tp0"], f64)
    w_tp1 = np.asarray(inputs["w_tp1"], f64)
    gamma = float(np.asarray(inputs["output_scale"]))

    alpha = 1.0 / C_SILU
    im, ik = INV_SQRT_M, INV_SQRT_K

    W0s = 0.5 * (w_tp0 + w_tp0.T) * TP_NORM / gamma
    W1s = 0.5 * (w_tp1 + w_tp1.T) * INV_SQRT_3 * TP_NORM / gamma
    lam0, Q0 = np.linalg.eigh(W0s)
    lam1, Q1 = np.linalg.eigh(W1s)

    wcat = np.zeros((128, NW), np.float16)
    bcat = np.zeros((128, NB), np.float32)
    for l, (w0, b0, w1) in enumerate(((w0_1, b0_1, w1_1), (w0_2, b0_2, w1_2))):
        wcat[:, _OFF_LS[l]:_OFF_LS[l] + 128] = (alpha * im * w0[:, :128]).astype(np.float16)
        wcat[:, _OFF_LG[l]:_OFF_LG[l] + 64] = (alpha * im / C_RELU * w0[:, 128:]).astype(np.float16)
        bd = ik * w1
        wcat[0:64, _OFF_BD[l]:_OFF_BD[l] + 64] = bd.astype(np.float16)
        wcat[64:128, _OFF_BD[l] + 64:_OFF_BD[l] + 128] = bd.astype(np.float16)
        bcat[:, l] = b0[:128].astype(np.float32)
        gate_b = (b0[128:] / C_RELU).astype(np.float32)
        bcat[0:64, 2 + l] = gate_b
        bcat[64:128, 2 + l] = gate_b
    wcat[:, _OFF_LR0:_OFF_LR0 + 64] = (alpha * im * (w0_o @ Q0)).astype(np.float16)
    lr1 = (ik * (w1_o @ Q1)).astype(np.float16)
    wcat[0:64, _OFF_LR1BD:_OFF_LR1BD + 64] = lr1
    wcat[64:128, _OFF_LR1BD + 64:_OFF_LR1BD + 128] = lr1
    # paired reduce weights: col 0 reduces the even-tile half, col 1 the odd half
    wcat[0:64, _OFF_RP] = lam0.astype(np.float16)
    wcat[64:128, _OFF_RP + 1] = lam0.astype(np.float16)
    wcat[0:64, _OFF_RQ] = lam1.astype(np.float16)
    wcat[64:128, _OFF_RQ + 1] = lam1.astype(np.float16)
    br0 = (Q0.T @ b0_o).astype(np.float32)
    bcat[0:64, 4] = br0
    bcat[64:128, 4] = br0
    return wcat, bcat


def _prep_x(x: np.ndarray, shift: np.ndarray, n_samp: int,
            n_cores: int = N_CORES) -> list[tuple[np.ndarray, np.ndarray]]:
    """Per-core feature-major fp16 arrays: xu [128, n], xv [128, 3n/2]."""
    xs_scale = np.float32(C_SILU)
    shift = np.asarray(shift, np.float32)
    n_pair = n_samp // 1024
    out = []
    for c in range(n_cores):
        blk = np.asarray(x[c * n_samp:(c + 1) * n_samp], np.float32) - shift
        xu = np.ascontiguousarray((blk[:, :128] * xs_scale).T.astype(np.float16))
        # vecs: [n, 64, 3] -> comp-major [3, 64, n]
        vv = blk[:, 128:].reshape(n_samp, 64, 3).transpose(2, 1, 0).astype(np.float16)
        # [3, 64, n] -> [3, 64, n_pair, 2, 512]; half 0 = even tile, 1 = odd
        vv = vv.reshape(3, 64, n_pair, 2, NT)
        xvh = np.empty((2, 64, n_pair, 3, NT), np.float16)
        xvh[0] = vv[:, :, :, 0, :].transpose(1, 2, 0, 3)   # even rows 0:64
        xvh[1] = vv[:, :, :, 1, :].transpose(1, 2, 0, 3)   # odd rows 64:128
        xv = xvh.reshape(128, n_pair * 3 * NT)
        out.append((xu, np.ascontiguousarray(xv)))
    return out


def _get_module():
    if "nc" not in _CACHE:
        _CACHE["nc"] = _build_module(NC_SAMP, NPAIR)
    return _CACHE["nc"]


def run(inputs: dict, trace: bool = False):
    """Run on 8 NeuronCores; returns (y [N,1] f32, BassKernelResults)."""
    from concourse import bass_utils
    from concourse.bass_interp import get_hw_module

    nc = _get_module()
    wcat, bcat = _prep_weights(inputs)
    xs = _prep_x(np.asarray(inputs["x"]), np.asarray(inputs["input_shift"]), NC_SAMP)
    in_maps = [
        {"xu": xs[c][0], "xv": xs[c][1], "wcat": wcat, "bcat": bcat}
        for c in range(N_CORES)
    ]

    old_m = nc.m
    nc.m = get_hw_module(nc.m)
    try:
        res = bass_utils.run_bass_kernel_spmd(
            nc,
            in_maps,
            core_ids=list(range(N_CORES)),
            trace=trace,
        )
    finally:
        nc.m = old_m

    # de-interleave: y dram is [2, NC/2]; row 0 = even tiles, row 1 = odd
    parts = []
    for c in range(N_CORES):
        yc = res.results[c]["y"]
        arr = np.empty((2 * NPAIR, NT), np.float32)
        arr[0::2] = yc[0].reshape(NPAIR, NT)
        arr[1::2] = yc[1].reshape(NPAIR, NT)
        parts.append(arr.reshape(-1))
    yfull = np.concatenate(parts)
    return yfull.astype(np.float32)[:, None], res


def kernel(**inputs) -> np.ndarray:
    y, _ = run(inputs, trace=False)
    return y


# revision 27
# speedup vs baseline: 1.2044x; 1.0253x over previous
"""Trainium2 Bass kernel for nn_CorrNet (e3nn-style equivariant MLP + tensor-product head).

Contract: kernel(**inputs) takes the FULL unsharded inputs (as produced by
setup_inputs()) and returns the FULL [N, 1] float32 output.

Strategy (pure data parallel over the atom axis N, 8 NeuronCores):
 - Host: fold every static scalar (1/sqrt(M), 1/sqrt(K), act norms, tp norm,
   output_scale, input_shift) into the weights; eigendecompose the symmetric
   tensor-product forms so the quadratic head becomes
   y = sum_e lam0_e (Q0^T zs)_e^2 + sum_{i,e} lam1_e (Q1^T zv_i)_e^2,
   i.e. pure matmuls + Square activations + a lambda-weighted partition
   reduction (one more matmul with a [128,2] stationary operand).
 - Host: re-layout x into feature-major fp16 arrays per core:
   xu [128, n] (0e block, pre-scaled by C_SILU) and xv [128, 3n/2]
   (1e block, per 1024-sample pair: 3 components x 512 cols, with the
   even tile's 64 rows on partitions 0:64 and the odd tile's on 64:128).
 - Device: inputs are bulk-DMAed into resident SBUF tiles up front (16
   chunked DMAs) and updated in place by the resnet; per 1024-sample
   iteration: 23 matmuls, silu/relu/square on ScalarE, gate multiplies +
   vector residual add on VectorE, scalar residual add on GpSimd, and the
   [2,512] result DMAed straight out of PSUM.

Everything is exact algebra up to fp16 storage rounding of activations and
weights; all accumulation is fp32.
"""

import numpy as np

# ---- problem constants (hardcoded per contest contract) ----
M, K, T = 128, 64, 64
N_TOTAL = 131072
N_CORES = 8
NC_SAMP = N_TOTAL // N_CORES  # 16384 samples per core
NT = 512                      # samples per tile
NPAIR = NC_SAMP // (2 * NT)   # 16 iterations of 1024 samples

C_SILU = 0.5964692111226791
C_RELU = 0.7071067811865186
INV_SQRT_M = float(1.0 / np.sqrt(M))
INV_SQRT_K = float(1.0 / np.sqrt(K))
INV_SQRT_3 = float(1.0 / np.sqrt(3.0))
TP_NORM = float(1.0 / np.sqrt(2.0 * T * T))

# weight-concat column offsets (fp16 [128, NW])
_OFF_LS = (0, 320)
_OFF_LG = (128, 448)
_OFF_BD = (192, 512)
_OFF_LR0 = 640
_OFF_LR1BD = 704
# paired-reduce lhsT columns [128, 2] each
_OFF_RP = 832   # [lam0; 0], [0; lam0]
_OFF_RQ = 834   # [lam1; 0], [0; lam1]
NW = 836
NB = 5  # f32 bias columns: BS1, BS2, BG1dup, BG2dup, BRAdup

# ---- engine assignment knobs ----
UADD_ENGINE = "dve"      # "gpsimd" | "dve"   (scalar-block residual add)
RELU_ENGINE = "scalar"   # "scalar" | "dve"   (gate relu)
YCOPY_ENGINE = "dve"     # "dve" | "scalar"   ([2,512] PSUM -> SBUF y stage)

_CACHE: dict = {}


def _build_module(n_samp: int, n_pair: int):
    """Build + compile the Bass/Tile module for one core (n_samp = n_pair*1024)."""
    from contextlib import ExitStack

    import concourse.bass as bass
    import concourse.tile as tile
    from concourse import bacc, mybir

    f16 = mybir.dt.float16
    f32 = mybir.dt.float32
    AF = mybir.ActivationFunctionType

    nc = bacc.Bacc(
        "TRN2",
        target_bir_lowering=False,
        debug=False,
        enable_asserts=False,
        num_devices=N_CORES,
    )
    nv = 3 * n_samp // 2
    xu = nc.dram_tensor("xu", [128, n_samp], f16, kind="ExternalInput").ap()
    xv = nc.dram_tensor("xv", [128, nv], f16, kind="ExternalInput").ap()
    wcat = nc.dram_tensor("wcat", [128, NW], f16, kind="ExternalInput").ap()
    bcat = nc.dram_tensor("bcat", [128, NB], f32, kind="ExternalInput").ap()
    y = nc.dram_tensor("y", [2, n_samp // 2], f32, kind="ExternalOutput").ap()

    with tile.TileContext(nc) as tc, ExitStack() as ctx:
        wpool = ctx.enter_context(tc.tile_pool(name="w", bufs=1))
        iopool = ctx.enter_context(tc.tile_pool(name="io", bufs=1))
        sbp = ctx.enter_context(tc.tile_pool(name="sb", bufs=2))
        psp = ctx.enter_context(tc.tile_pool(name="ps", bufs=1, space="PSUM"))
        ypool = ctx.enter_context(tc.tile_pool(name="yp", bufs=1))

        W = wpool.tile([128, NW], f16, tag="W")
        nc.sync.dma_start(W[:], wcat[:])
        B = wpool.tile([128, NB], f32, tag="B")
        nc.sync.dma_start(B[:], bcat[:])

        XU = iopool.tile([128, n_samp], f16, tag="XU")
        XV = iopool.tile([128, nv], f16, tag="XV")
        # chunked bulk preload (2 pairs' worth per chunk)
        ck_u, ck_v = 2048, 3072
        for k in range(n_samp // ck_u):
            nc.sync.dma_start(XU[:, k * ck_u:(k + 1) * ck_u], xu[:, k * ck_u:(k + 1) * ck_u])
            nc.sync.dma_start(XV[:, k * ck_v:(k + 1) * ck_v], xv[:, k * ck_v:(k + 1) * ck_v])

        LS =[W[:, _OFF_LS[0]:_OFF_LS[0] + 128], W[:, _OFF_LS[1]:_OFF_LS[1] + 128]]
        LG = [W[:, _OFF_LG[0]:_OFF_LG[0] + 64], W[:, _OFF_LG[1]:_OFF_LG[1] + 64]]
        BD = [W[:, _OFF_BD[0]:_OFF_BD[0] + 128], W[:, _OFF_BD[1]:_OFF_BD[1] + 128]]
        LR0 = W[:, _OFF_LR0:_OFF_LR0 + 64]
        LR1BD = W[:, _OFF_LR1BD:_OFF_LR1BD + 128]
        RP = W[:, _OFF_RP:_OFF_RP + 2]
        RQ = W[:, _OFF_RQ:_OFF_RQ + 2]
        BS = [B[:, 0:1], B[:, 1:2]]
        BG = [B[:, 2:3], B[:, 3:4]]
        BRA = B[:, 4:5]

        def uadd(u_sl, ts_t):
            if UADD_ENGINE == "gpsimd":
                nc.gpsimd.tensor_add(u_sl, u_sl, ts_t[:])
            else:
                nc.vector.tensor_add(u_sl, u_sl, ts_t[:])

        # deferred reduce state from the previous iteration (1-iter skew)
        pend = None

        def emit_reduce(pd):
            # reduce of a prior iteration: 4 chained matmuls into a dedicated
            # y PSUM bank, then a [2,512] copy to SBUF and a direct DMA out
            p0, sqP0, sqV0 = pd
            py = psp.tile([2, NT], f32, tag="y")
            nc.tensor.matmul(py[:], RP, sqP0[:], start=True, stop=False)
            nc.tensor.matmul(py[:], RQ, sqV0[:, 0:NT], start=False, stop=False)
            nc.tensor.matmul(py[:], RQ, sqV0[:, NT:1024], start=False, stop=False)
            nc.tensor.matmul(py[:], RQ, sqV0[:, 1024:1536], start=False, stop=True)
            yt = sbp.tile([2, NT], f32, tag="yt")
            if YCOPY_ENGINE == "scalar":
                nc.scalar.copy(yt[:], py[:])
            else:
                nc.vector.tensor_copy(yt[:], py[:])
            nc.sync.dma_start(y[0:2, bass.ts(p0, NT)], yt[:])

        for p in range(n_pair):
            u = XU[:, 1024 * p:1024 * (p + 1)]
            ue = u[:, 0:NT]
            uo = u[:, NT:2 * NT]
            v = XV[:, 1536 * p:1536 * (p + 1)]
            v01 = v[:, 0:1024]
            v2 = v[:, 1024:1536]

            for l in range(2):
                sp = psp.tile([128, 1024], f32, tag="sp")
                nc.tensor.matmul(sp[:, 0:NT], LS[l], ue, start=True, stop=True)
                nc.tensor.matmul(sp[:, NT:2 * NT], LS[l], uo, start=True, stop=True)
                pg = psp.tile([128, NT], f32, tag="g")
                nc.tensor.matmul(pg[0:64, :], LG[l], ue, start=True, stop=True)
                nc.tensor.matmul(
                    pg[64:128, :], LG[l], uo,
                    start=True, stop=True, tile_position=(0, 64),
                )
                pv01 = psp.tile([128, 1024], f32, tag="v01")
                nc.tensor.matmul(pv01[:, 0:NT], BD[l], v01[:, 0:NT], start=True, stop=True)
                nc.tensor.matmul(pv01[:, NT:1024], BD[l], v01[:, NT:1024], start=True, stop=True)
                pv2 = psp.tile([128, NT], f32, tag="v2")
                nc.tensor.matmul(pv2[:], BD[l], v2[:], start=True, stop=True)

                if l == 0 and pend is not None:
                    emit_reduce(pend)   # PE filler while relu/silu run

                # relu first: it unblocks the three gate multiplies
                g01 = sbp.tile([128, NT], f16, tag="g01")
                nc.scalar.activation(g01[:], pg[:], AF.Relu, bias=BG[l])
                ts = sbp.tile([128, 1024], f16, tag="ts")
                nc.scalar.activation(ts[:], sp[:], AF.Silu, bias=BS[l])

                tv = sbp.tile([128, 1536], f16, tag="tv")
                nc.vector.tensor_mul(tv[:, 0:NT], pv01[:, 0:NT], g01[:])
                uadd(u, ts)
                nc.vector.tensor_mul(tv[:, NT:1024], pv01[:, NT:1024], g01[:])
                nc.vector.tensor_mul(tv[:, 1024:1536], pv2[:], g01[:])
                nc.vector.tensor_add(v, v, tv[:])

            # output head
            pP = psp.tile([128, NT], f32, tag="P")
            nc.tensor.matmul(pP[0:64, :], LR0, ue, start=True, stop=True)
            nc.tensor.matmul(
                pP[64:128, :], LR0, uo,
                start=True, stop=True, tile_position=(0, 64),
            )
            sqP = sbp.tile([128, NT], f16, tag="sqP")
            nc.scalar.activation(sqP[:], pP[:], AF.Square, bias=BRA)

            pq01 = psp.tile([128, 1024], f32, tag="v01")
            nc.tensor.matmul(pq01[:, 0:NT], LR1BD, v01[:, 0:NT], start=True, stop=True)
            nc.tensor.matmul(pq01[:, NT:1024], LR1BD, v01[:, NT:1024], start=True, stop=True)
            pq2 = psp.tile([128, NT], f32, tag="v2")
            nc.tensor.matmul(pq2[:], LR1BD, v2[:], start=True, stop=True)
            sqV = sbp.tile([128, 1536], f16, tag="sqV")
            nc.scalar.activation(sqV[:, 0:1024], pq01[:], AF.Square)
            nc.scalar.activation(sqV[:, 1024:1536], pq2[:], AF.Square)

            pend = (p, sqP, sqV)

        emit_reduce(pend)

    nc.compile()
    return nc


def _prep_weights(inputs: dict) -> tuple[np.ndarray, np.ndarray]:
    """Fold all scalars into fp16 stationary operands + f32 bias columns."""
    f64 = np.float64
    w0_1 = np.asarray(inputs["w0_1"], f64)
    b0_1 = np.asarray(inputs["b0_1"], f64)
    w1_1 = np.asarray(inputs["w1_1"], f64)
    w0_2 = np.asarray(inputs["w0_2"], f64)
    b0_2 = np.asarray(inputs["b0_2"], f64)
    w1_2 = np.asarray(inputs["w1_2"], f64)
    w0_o = np.asarray(inputs["w0_o"], f64)
    b0_o = np.asarray(inputs["b0_o"], f64)
    w1_o = np.asarray(inputs["w1_o"], f64)
    w_tp0 = np.asarray(inputs["w_tp0"], f64)
    w_tp1 = np.asarray(inputs["w_tp1"], f64)
    gamma = float(np.asarray(inputs["output_scale"]))

    alpha = 1.0 / C_SILU
    im, ik = INV_SQRT_M, INV_SQRT_K

    W0s = 0.5 * (w_tp0 + w_tp0.T) * TP_NORM / gamma
    W1s = 0.5 * (w_tp1 + w_tp1.T) * INV_SQRT_3 * TP_NORM / gamma
    lam0, Q0 = np.linalg.eigh(W0s)
    lam1, Q1 = np.linalg.eigh(W1s)

    wcat = np.zeros((128, NW), np.float16)
    bcat = np.zeros((128, NB), np.float32)
    for l, (w0, b0, w1) in enumerate(((w0_1, b0_1, w1_1), (w0_2, b0_2, w1_2))):
        wcat[:, _OFF_LS[l]:_OFF_LS[l] + 128] = (alpha * im * w0[:, :128]).astype(np.float16)
        wcat[:, _OFF_LG[l]:_OFF_LG[l] + 64] = (alpha * im / C_RELU * w0[:, 128:]).astype(np.float16)
        bd = ik * w1
        wcat[0:64, _OFF_BD[l]:_OFF_BD[l] + 64] = bd.astype(np.float16)
        wcat[64:128, _OFF_BD[l] + 64:_OFF_BD[l] + 128] = bd.astype(np.float16)
        bcat[:, l] = b0[:128].astype(np.float32)
        gate_b = (b0[128:] / C_RELU).astype(np.float32)
        bcat[0:64, 2 + l] = gate_b
        bcat[64:128, 2 + l] = gate_b
    wcat[:, _OFF_LR0:_OFF_LR0 + 64] = (alpha * im * (w0_o @ Q0)).astype(np.float16)
    lr1 = (ik * (w1_o @ Q1)).astype(np.float16)
    wcat[0:64, _OFF_LR1BD:_OFF_LR1BD + 64] = lr1
    wcat[64:128, _OFF_LR1BD + 64:_OFF_LR1BD + 128] = lr1
    # paired reduce weights: col 0 reduces the even-tile half, col 1 the odd half
    wcat[0:64, _OFF_RP] = lam0.astype(np.float16)
    wcat[64:128, _OFF_RP + 1] = lam0.astype(np.float16)
    wcat[0:64, _OFF_RQ] = lam1.astype(np.float16)
    wcat[64:128, _OFF_RQ + 1] = lam1.astype(np.float16)
    br0 = (Q0.T @ b0_o).astype(np.float32)
    bcat[0:64, 4] = br0
    bcat[64:128, 4] = br0
    return wcat, bcat


def _prep_x(x: np.ndarray, shift: np.ndarray, n_samp: int,
            n_cores: int = N_CORES) -> list[tuple[np.ndarray, np.ndarray]]:
    """Per-core feature-major fp16 arrays: xu [128, n], xv [128, 3n/2]."""
    xs_scale = np.float32(C_SILU)
    shift = np.asarray(shift, np.float32)
    n_pair = n_samp // 1024
    out = []
    for c in range(n_cores):
        blk = np.asarray(x[c * n_samp:(c + 1) * n_samp], np.float32) - shift
        xu = np.ascontiguousarray((blk[:, :128] * xs_scale).T.astype(np.float16))
        # vecs: [n, 64, 3] -> comp-major [3, 64, n]
        vv = blk[:, 128:].reshape(n_samp, 64, 3).transpose(2, 1, 0).astype(np.float16)
        # [3, 64, n] -> [3, 64, n_pair, 2, 512]; half 0 = even tile, 1 = odd
        vv = vv.reshape(3, 64, n_pair, 2, NT)
        xvh = np.empty((2, 64, n_pair, 3, NT), np.float16)
        xvh[0] = vv[:, :, :, 0, :].transpose(1, 2, 0, 3)   # even rows 0:64
        xvh[1] = vv[:, :, :, 1, :].transpose(1, 2, 0, 3)   # odd rows 64:128
        xv = xvh.reshape(128, n_pair * 3 * NT)
        out.append((xu, np.ascontiguousarray(xv)))
    return out


def _get_module():
    if "nc" not in _CACHE:
        _CACHE["nc"] = _build_module(NC_SAMP, NPAIR)
    return _CACHE["nc"]


def run(inputs: dict, trace: bool = False):
    """Run on 8 NeuronCores; returns (y [N,1] f32, BassKernelResults)."""
    from concourse import bass_utils
    from concourse.bass_interp import get_hw_module

    nc = _get_module()
    wcat, bcat = _prep_weights(inputs)
    xs = _prep_x(np.asarray(inputs["x"]), np.asarray(inputs["input_shift"]), NC_SAMP)
    in_maps = [
        {"xu": xs[c][0], "xv": xs[c][1], "wcat": wcat, "bcat": bcat}
        for c in range(N_CORES)
    ]

    old_m = nc.m
    nc.m = get_hw_module(nc.m)
    try:
        res = bass_utils.run_bass_kernel_spmd(
            nc,
            in_maps,
            core_ids=list(range(N_CORES)),
            trace=trace,
        )
    finally:
        nc.m = old_m

    # de-interleave: y dram is [2, NC/2]; row 0 = even tiles, row 1 = odd
    parts = []
    for c in range(N_CORES):
        yc = res.results[c]["y"]
        arr = np.empty((2 * NPAIR, NT), np.float32)
        arr[0::2] = yc[0].reshape(NPAIR, NT)
        arr[1::2] = yc[1].reshape(NPAIR, NT)
        parts.append(arr.reshape(-1))
    yfull = np.concatenate(parts)
    return yfull.astype(np.float32)[:, None], res


def kernel(**inputs) -> np.ndarray:
    y, _ = run(inputs, trace=False)
    return y


# revision 28
# speedup vs baseline: 1.2995x; 1.0789x over previous
"""Trainium2 Bass kernel for nn_CorrNet (e3nn-style equivariant MLP + tensor-product head).

Contract: kernel(**inputs) takes the FULL unsharded inputs (as produced by
setup_inputs()) and returns the FULL [N, 1] float32 output.

Strategy (pure data parallel over the atom axis N, 8 NeuronCores):
 - Host: fold every static scalar (1/sqrt(M), 1/sqrt(K), act norms, tp norm,
   output_scale, input_shift) into the weights; eigendecompose the symmetric
   tensor-product forms so the quadratic head becomes
   y = sum_e lam0_e (Q0^T zs)_e^2 + sum_{i,e} lam1_e (Q1^T zv_i)_e^2,
   i.e. pure matmuls + Square activations + a lambda-weighted partition
   reduction (one more matmul with a [128,2] stationary operand).
 - Host: re-layout x into feature-major fp16 arrays per core:
   xu [128, n] (0e block, pre-scaled by C_SILU) and xv [128, 3n/2]
   (1e block, per 1024-sample pair: 3 components x 512 cols, with the
   even tile's 64 rows on partitions 0:64 and the odd tile's on 64:128).
 - Device: inputs are bulk-DMAed into resident SBUF tiles up front (16
   chunked DMAs) and updated in place by the resnet; per 1024-sample
   iteration: 23 matmuls, silu/relu/square on ScalarE, gate multiplies +
   vector residual add on VectorE, scalar residual add on GpSimd, and the
   [2,512] result DMAed straight out of PSUM.

Everything is exact algebra up to fp16 storage rounding of activations and
weights; all accumulation is fp32.
"""

import numpy as np

# ---- problem constants (hardcoded per contest contract) ----
M, K, T = 128, 64, 64
N_TOTAL = 131072
N_CORES = 8
NC_SAMP = N_TOTAL // N_CORES  # 16384 samples per core
NT = 512                      # samples per tile
NPAIR = NC_SAMP // (2 * NT)   # 16 iterations of 1024 samples

C_SILU = 0.5964692111226791
C_RELU = 0.7071067811865186
INV_SQRT_M = float(1.0 / np.sqrt(M))
INV_SQRT_K = float(1.0 / np.sqrt(K))
INV_SQRT_3 = float(1.0 / np.sqrt(3.0))
TP_NORM = float(1.0 / np.sqrt(2.0 * T * T))

# weight-concat column offsets (fp16 [128, NW])
_OFF_LS = (0, 320)
_OFF_LG = (128, 448)
_OFF_BD = (192, 512)
_OFF_LR0 = 640
_OFF_LR1BD = 704
# paired-reduce lhsT columns [128, 2] each
_OFF_RP = 832   # [lam0; 0], [0; lam0]
_OFF_RQ = 834   # [lam1; 0], [0; lam1]
NW = 836
NB = 5  # f32 bias columns: BS1, BS2, BG1dup, BG2dup, BRAdup

# ---- engine assignment knobs ----
UADD_ENGINE = "dve"      # "gpsimd" | "dve"   (scalar-block residual add)
RELU_ENGINE = "scalar"   # "scalar" | "dve"   (gate relu)
YCOPY_ENGINE = "dve"     # "dve" | "scalar"   ([2,512] PSUM -> SBUF y stage)

_CACHE: dict = {}


def _build_module(n_samp: int, n_pair: int):
    """Build + compile the Bass/Tile module for one core (n_samp = n_pair*1024)."""
    from contextlib import ExitStack

    import concourse.bass as bass
    import concourse.tile as tile
    from concourse import bacc, mybir

    f16 = mybir.dt.float16
    f32 = mybir.dt.float32
    AF = mybir.ActivationFunctionType

    nc = bacc.Bacc(
        "TRN2",
        target_bir_lowering=False,
        debug=False,
        enable_asserts=False,
        num_devices=N_CORES,
    )
    nv = 3 * n_samp // 2
    xu = nc.dram_tensor("xu", [128, n_samp], f16, kind="ExternalInput").ap()
    xv = nc.dram_tensor("xv", [128, nv], f16, kind="ExternalInput").ap()
    wcat = nc.dram_tensor("wcat", [128, NW], f16, kind="ExternalInput").ap()
    bcat = nc.dram_tensor("bcat", [128, NB], f32, kind="ExternalInput").ap()
    y = nc.dram_tensor("y", [2, n_samp // 2], f32, kind="ExternalOutput").ap()

    with tile.TileContext(nc) as tc, ExitStack() as ctx:
        wpool = ctx.enter_context(tc.tile_pool(name="w", bufs=1))
        iopool = ctx.enter_context(tc.tile_pool(name="io", bufs=1))
        sbp = ctx.enter_context(tc.tile_pool(name="sb", bufs=2))
        psp = ctx.enter_context(tc.tile_pool(name="ps", bufs=1, space="PSUM"))
        ypool = ctx.enter_context(tc.tile_pool(name="yp", bufs=1))

        W = wpool.tile([128, NW], f16, tag="W")
        nc.sync.dma_start(W[:], wcat[:])
        B = wpool.tile([128, NB], f32, tag="B")
        nc.sync.dma_start(B[:], bcat[:])

        XU = iopool.tile([128, n_samp], f16, tag="XU")
        XV = iopool.tile([128, nv], f16, tag="XV")
        # chunked bulk preload (2 pairs' worth per chunk)
        ck_u, ck_v = 2048, 3072
        for k in range(n_samp // ck_u):
            nc.sync.dma_start(XU[:, k * ck_u:(k + 1) * ck_u], xu[:, k * ck_u:(k + 1) * ck_u])
            nc.sync.dma_start(XV[:, k * ck_v:(k + 1) * ck_v], xv[:, k * ck_v:(k + 1) * ck_v])

        LS =[W[:, _OFF_LS[0]:_OFF_LS[0] + 128], W[:, _OFF_LS[1]:_OFF_LS[1] + 128]]
        LG = [W[:, _OFF_LG[0]:_OFF_LG[0] + 64], W[:, _OFF_LG[1]:_OFF_LG[1] + 64]]
        BD = [W[:, _OFF_BD[0]:_OFF_BD[0] + 128], W[:, _OFF_BD[1]:_OFF_BD[1] + 128]]
        LR0 = W[:, _OFF_LR0:_OFF_LR0 + 64]
        LR1BD = W[:, _OFF_LR1BD:_OFF_LR1BD + 128]
        RP = W[:, _OFF_RP:_OFF_RP + 2]
        RQ = W[:, _OFF_RQ:_OFF_RQ + 2]
        BS = [B[:, 0:1], B[:, 1:2]]
        BG = [B[:, 2:3], B[:, 3:4]]
        BRA = B[:, 4:5]

        def uadd(u_sl, ts_t):
            if UADD_ENGINE == "gpsimd":
                nc.gpsimd.tensor_add(u_sl, u_sl, ts_t[:])
            else:
                nc.vector.tensor_add(u_sl, u_sl, ts_t[:])

        # one-iteration software pipeline: the head (LR0/Q matmuls, squares,
        # lambda-reduce, y copy-out) of iteration p-1 executes inside
        # iteration p, filling every engine while p's layer chain resolves.
        pend = None

        def emit_head(pd):
            p0, ue0, uo0, v0 = pd
            # LR0 -> gA bank (after relu-L1(p) drains it)
            pP = psp.tile([128, NT], f32, tag="gA")
            nc.tensor.matmul(pP[0:64, :], LR0, ue0, start=True, stop=True)
            nc.tensor.matmul(
                pP[64:128, :], LR0, uo0,
                start=True, stop=True, tile_position=(0, 64),
            )
            # Q -> contiguous v bank
            pq = psp.tile([128, 1536], f32, tag="v")
            nc.tensor.matmul(pq[:, 0:NT], LR1BD, v0[:, 0:NT], start=True, stop=True)
            nc.tensor.matmul(pq[:, NT:1024], LR1BD, v0[:, NT:1024], start=True, stop=True)
            nc.tensor.matmul(pq[:, 1024:1536], LR1BD, v0[:, 1024:1536], start=True, stop=True)
            sqP = sbp.tile([128, NT], f16, tag="sqP")
            nc.scalar.activation(sqP[:], pP[:], AF.Square, bias=BRA)
            sqV = sbp.tile([128, 1536], f16, tag="sqV")
            nc.scalar.activation(sqV[:], pq[:], AF.Square)
            # lambda-weighted partition reduce -> [2, NT], then copy + DMA out
            py = psp.tile([2, NT], f32, tag="y")
            nc.tensor.matmul(py[:], RP, sqP[:], start=True, stop=False)
            nc.tensor.matmul(py[:], RQ, sqV[:, 0:NT], start=False, stop=False)
            nc.tensor.matmul(py[:], RQ, sqV[:, NT:1024], start=False, stop=False)
            nc.tensor.matmul(py[:], RQ, sqV[:, 1024:1536], start=False, stop=True)
            yt = sbp.tile([2, NT], f32, tag="yt")
            if p0 % 2 == 0:
                nc.scalar.copy(yt[:], py[:])
            else:
                nc.vector.tensor_copy(yt[:], py[:])
            nc.sync.dma_start(y[0:2, bass.ts(p0, NT)], yt[:])

        for p in range(n_pair):
            u = XU[:, 1024 * p:1024 * (p + 1)]
            ue = u[:, 0:NT]
            uo = u[:, NT:2 * NT]
            v = XV[:, 1536 * p:1536 * (p + 1)]

            for l in range(2):
                sp = psp.tile([128, 1024], f32, tag="sp")
                nc.tensor.matmul(sp[:, 0:NT], LS[l], ue, start=True, stop=True)
                nc.tensor.matmul(sp[:, NT:2 * NT], LS[l], uo, start=True, stop=True)
                pg = psp.tile([128, NT], f32, tag="gA" if l == 0 else "gB")
                nc.tensor.matmul(pg[0:64, :], LG[l], ue, start=True, stop=True)
                nc.tensor.matmul(
                    pg[64:128, :], LG[l], uo,
                    start=True, stop=True, tile_position=(0, 64),
                )
                pv = psp.tile([128, 1536], f32, tag="v")
                nc.tensor.matmul(pv[:, 0:NT], BD[l], v[:, 0:NT], start=True, stop=True)
                nc.tensor.matmul(pv[:, NT:1024], BD[l], v[:, NT:1024], start=True, stop=True)
                nc.tensor.matmul(pv[:, 1024:1536], BD[l], v[:, 1024:1536], start=True, stop=True)

                # relu first: it unblocks the gate multiply
                g01 = sbp.tile([128, NT], f16, tag="g01")
                nc.scalar.activation(g01[:], pg[:], AF.Relu, bias=BG[l])
                ts = sbp.tile([128, 1024], f16, tag="ts")
                nc.scalar.activation(ts[:], sp[:], AF.Silu, bias=BS[l])

                tv = sbp.tile([128, 1536], f16, tag="tv")
                nc.vector.tensor_mul(
                    tv[:], pv[:].rearrange("p (c n) -> p c n", c=3),
                    g01[:].unsqueeze(1).broadcast_to((128, 3, NT)),
                )
                uadd(u, ts)
                nc.vector.tensor_add(v, v, tv[:])

                if l == 0 and pend is not None:
                    emit_head(pend)   # previous iteration's head: PE filler

            pend = (p, ue, uo, v)

        emit_head(pend)

    nc.compile()
    return nc


def _prep_weights(inputs: dict) -> tuple[np.ndarray, np.ndarray]:
    """Fold all scalars into fp16 stationary operands + f32 bias columns."""
    f64 = np.float64
    w0_1 = np.asarray(inputs["w0_1"], f64)
    b0_1 = np.asarray(inputs["b0_1"], f64)
    w1_1 = np.asarray(inputs["w1_1"], f64)
    w0_2 = np.asarray(inputs["w0_2"], f64)
    b0_2 = np.asarray(inputs["b0_2"], f64)
    w1_2 = np.asarray(inputs["w1_2"], f64)
    w0_o = np.asarray(inputs["w0_o"], f64)
    b0_o = np.asarray(inputs["b0_o"], f64)
    w1_o = np.asarray(inputs["w1_o"], f64)
    w_tp0 = np.asarray(inputs["w_tp0"], f64)
    w_tp1 = np.asarray(inputs["w_tp1"], f64)
    gamma = float(np.asarray(inputs["output_scale"]))

    alpha = 1.0 / C_SILU
    im, ik = INV_SQRT_M, INV_SQRT_K

    W0s = 0.5 * (w_tp0 + w_tp0.T) * TP_NORM / gamma
    W1s = 0.5 * (w_tp1 + w_tp1.T) * INV_SQRT_3 * TP_NORM / gamma
    lam0, Q0 = np.linalg.eigh(W0s)
    lam1, Q1 = np.linalg.eigh(W1s)

    wcat = np.zeros((128, NW), np.float16)
    bcat = np.zeros((128, NB), np.float32)
    for l, (w0, b0, w1) in enumerate(((w0_1, b0_1, w1_1), (w0_2, b0_2, w1_2))):
        wcat[:, _OFF_LS[l]:_OFF_LS[l] + 128] = (alpha * im * w0[:, :128]).astype(np.float16)
        wcat[:, _OFF_LG[l]:_OFF_LG[l] + 64] = (alpha * im / C_RELU * w0[:, 128:]).astype(np.float16)
        bd = ik * w1
        wcat[0:64, _OFF_BD[l]:_OFF_BD[l] + 64] = bd.astype(np.float16)
        wcat[64:128, _OFF_BD[l] + 64:_OFF_BD[l] + 128] = bd.astype(np.float16)
        bcat[:, l] = b0[:128].astype(np.float32)
        gate_b = (b0[128:] / C_RELU).astype(np.float32)
        bcat[0:64, 2 + l] = gate_b
        bcat[64:128, 2 + l] = gate_b
    wcat[:, _OFF_LR0:_OFF_LR0 + 64] = (alpha * im * (w0_o @ Q0)).astype(np.float16)
    lr1 = (ik * (w1_o @ Q1)).astype(np.float16)
    wcat[0:64, _OFF_LR1BD:_OFF_LR1BD + 64] = lr1
    wcat[64:128, _OFF_LR1BD + 64:_OFF_LR1BD + 128] = lr1
    # paired reduce weights: col 0 reduces the even-tile half, col 1 the odd half
    wcat[0:64, _OFF_RP] = lam0.astype(np.float16)
    wcat[64:128, _OFF_RP + 1] = lam0.astype(np.float16)
    wcat[0:64, _OFF_RQ] = lam1.astype(np.float16)
    wcat[64:128, _OFF_RQ + 1] = lam1.astype(np.float16)
    br0 = (Q0.T @ b0_o).astype(np.float32)
    bcat[0:64, 4] = br0
    bcat[64:128, 4] = br0
    return wcat, bcat


def _prep_x(x: np.ndarray, shift: np.ndarray, n_samp: int,
            n_cores: int = N_CORES) -> list[tuple[np.ndarray, np.ndarray]]:
    """Per-core feature-major fp16 arrays: xu [128, n], xv [128, 3n/2]."""
    xs_scale = np.float32(C_SILU)
    shift = np.asarray(shift, np.float32)
    n_pair = n_samp // 1024
    out = []
    for c in range(n_cores):
        blk = np.asarray(x[c * n_samp:(c + 1) * n_samp], np.float32) - shift
        xu = np.ascontiguousarray((blk[:, :128] * xs_scale).T.astype(np.float16))
        # vecs: [n, 64, 3] -> comp-major [3, 64, n]
        vv = blk[:, 128:].reshape(n_samp, 64, 3).transpose(2, 1, 0).astype(np.float16)
        # [3, 64, n] -> [3, 64, n_pair, 2, 512]; half 0 = even tile, 1 = odd
        vv = vv.reshape(3, 64, n_pair, 2, NT)
        xvh = np.empty((2, 64, n_pair, 3, NT), np.float16)
        xvh[0] = vv[:, :, :, 0, :].transpose(1, 2, 0, 3)   # even rows 0:64
        xvh[1] = vv[:, :, :, 1, :].transpose(1, 2, 0, 3)   # odd rows 64:128
        xv = xvh.reshape(128, n_pair * 3 * NT)
        out.append((xu, np.ascontiguousarray(xv)))
    return out


def _get_module():
    if "nc" not in _CACHE:
        _CACHE["nc"] = _build_module(NC_SAMP, NPAIR)
    return _CACHE["nc"]


def run(inputs: dict, trace: bool = False):
    """Run on 8 NeuronCores; returns (y [N,1] f32, BassKernelResults)."""
    from concourse import bass_utils
    from concourse.bass_interp import get_hw_module

    nc = _get_module()
    wcat, bcat = _prep_weights(inputs)
    xs = _prep_x(np.asarray(inputs["x"]), np.asarray(inputs["input_shift"]), NC_SAMP)
    in_maps = [
        {"xu": xs[c][0], "xv": xs[c][1], "wcat": wcat, "bcat": bcat}
        for c in range(N_CORES)
    ]

    old_m = nc.m
    nc.m = get_hw_module(nc.m)
    try:
        res = bass_utils.run_bass_kernel_spmd(
            nc,
            in_maps,
            core_ids=list(range(N_CORES)),
            trace=trace,
        )
    finally:
        nc.m = old_m

    # de-interleave: y dram is [2, NC/2]; row 0 = even tiles, row 1 = odd
    parts = []
    for c in range(N_CORES):
        yc = res.results[c]["y"]
        arr = np.empty((2 * NPAIR, NT), np.float32)
        arr[0::2] = yc[0].reshape(NPAIR, NT)
        arr[1::2] = yc[1].reshape(NPAIR, NT)
        parts.append(arr.reshape(-1))
    yfull = np.concatenate(parts)
    return yfull.astype(np.float32)[:, None], res


def kernel(**inputs) -> np.ndarray:
    y, _ = run(inputs, trace=False)
    return y
